# revision 4
# baseline (speedup 1.0000x reference)
"""Trainium2 Bass kernel for a 2-layer spiking (Synaptic) critic network.

Math (per batch row, T=8 steps, H=128, reset-by-subtract from previous spike):
    cur1 = state @ w_fc1.T
    syn1 = a1*syn1 + cur1 + spk1 @ w_rec1.T ; mem1 = b1*mem1 + syn1 - thr1*spk1_prev
    spk1 = (mem1 > thr1) ; layer 2 analogous with inputs spk1 @ w_fc2.T + spk2 @ w_rec2.T
    out_mean = tanh(mean_t(spk2) @ w_mean.T); out_std = 1.9*sigmoid(.. @ w_std.T + 2) + .1

Formulation (pure data parallel, 8 cores x 8192 rows, hidden on the 128
partitions, batch chunked into CB=512 columns, 16 chunks/core, 3 in flight):

  Work in the a^-t scaled domain so the synaptic accumulator stays resident
  in PSUM for all 8 steps with constant recurrent weights:
    A_t  = sum_{tau<=t} a^-tau (cur_tau + rec-input_tau)   (PSUM, PE-accumulated)
    M_t  = A_t + (b/a)*M_{t-1} - S_{t-1}
    S_t  = (M_t > thr*a^-t) * thr*a^-(t+1)
  The stored spike S carries the a^-(t+1) scale, which makes the recurrent
  matmul weights step-independent; the small feedforward weights get 8
  host-prescaled copies.

Engine assignment (measured per-[128,512]-tile costs in ns):
  PE   : 5 matmuls/chunk-step @216 (rec1, fc1-inject, fc2, rec2, head)
  ACT  : 2 PSUM drains @585 - z1/z2 written DIRECTLY into the membrane tile
  DVE  : W=M-S subtract @337 (STT @604 when b!=a), spike threshold TS @204
  SWDGE: membrane add M += W via gpsimd-issued DMA-accumulate (issue ~630,
         transfer ~390 on the DMA queues, off all compute engines)
  GPSIMD tensor ops are BANNED: the Q7 cores share an SBUF port with the
  vector engine and concurrent gpsimd tensor_tensor degrades every DVE op
  2-3x (measured).  The output head fuses tanh via 2*sigmoid(2x)-1 so one
  [2,512] activation + one per-partition-scalar TS serves both outputs.

Scheduling: plan-then-emit.  Ops are planned chronologically with a small
dependency tracker (per-buffer last-writer/readers -> cross-engine waits as
standalone wait_ge, coalesced via per-engine high-water marks), then each
engine block replays its list.  Raw Bass (no Tile): this walrus build
rejects instructions with >1 attached semaphore wait.
"""

import os
from contextlib import ExitStack

import numpy as np

N_CORES = 8
B_TOTAL = 65536
BC = B_TOTAL // N_CORES  # 8192 rows per core
CB = 512                 # batch-column chunk (one PSUM bank)
NCHUNK = BC // CB        # 16
G = 3                    # chunks interleaved in flight
T = 8
H = 128
SD = 6

GROUPS = [list(range(g, min(g + G, NCHUNK))) for g in range(0, NCHUNK, G)]

_CACHE: dict = {}


class _Sched:
    """Chronological planner: op closures + auto cross-engine waits."""

    ENGINES = ("pe", "dve", "act", "gps", "sp")

    def __init__(self):
        self.plan = {e: [] for e in self.ENGINES}
        self.count = {e: 0 for e in self.ENGINES}   # sem value after op k = k
        self.dma_count = {"sw": 0, "hw": 0}         # completions inc by 16
        self.hwm = {e: {} for e in self.ENGINES}    # per-engine seen sem values
        self.buf = {}                               # name -> {"w": ev, "r": [ev]}

    # events are (sem_name, value_needed)
    def _deps_read(self, name):
        b = self.buf.get(name)
        return [b["w"]] if b and b["w"] else []

    def _deps_write(self, name):
        b = self.buf.get(name)
        if not b:
            return []
        out = [b["w"]] if b["w"] else []
        return out + b["r"]

    def op(self, engine, fn, reads=(), writes=(), rmw=(), dma=None):
        """Plan one op.  engine: issuing engine.  dma: None for compute ops,
        "sw"/"hw" for DMA ops whose completion is a sem inc of 16 on the
        s_sw/s_dma semaphore (issue order still follows `engine`'s stream).
        rmw: buffers both read and written (accumulate)."""
        deps = []
        for n in reads:
            deps += self._deps_read(n)
        for n in writes:
            deps += self._deps_write(n)
        for n in rmw:
            deps += self._deps_write(n)
        # coalesce: keep max value per sem, drop already-satisfied
        need = {}
        for sem, val in deps:
            if sem == engine and dma is None:
                continue  # same-engine program order
            need[sem] = max(need.get(sem, 0), val)
        hw = self.hwm[engine]
        for sem, val in sorted(need.items()):
            if hw.get(sem, 0) < val:
                self.plan[engine].append(("wait", sem, val))
                hw[sem] = val
        if dma is None:
            self.count[engine] += 1
            ev = (engine, self.count[engine])
            self.plan[engine].append(("op", fn, engine, self.count[engine]))
        else:
            self.dma_count[dma] += 16
            ev = (dma, self.dma_count[dma])
            self.plan[engine].append(("op", fn, dma, self.dma_count[dma]))
        for n in writes:
            self.buf[n] = {"w": ev, "r": []}
        for n in rmw:
            b = self.buf.setdefault(n, {"w": None, "r": []})
            b["w"] = ev
            b["r"] = []
        for n in reads:
            b = self.buf.setdefault(n, {"w": None, "r": []})
            b["r"].append(ev)
        return ev


def _build(scal):
    import concourse.bass as bass
    import concourse.mybir as mybir

    a1, b1, thr1 = scal["a1"], scal["b1"], scal["thr1"]
    a2, b2, thr2 = scal["a2"], scal["b2"], scal["thr2"]
    f32 = mybir.dt.float32
    bf16 = mybir.dt.bfloat16
    Alu = mybir.AluOpType
    Act = mybir.ActivationFunctionType

    ba1 = b1 / a1
    ba2 = b2 / a2
    simple1 = abs(ba1 - 1.0) < 1e-12
    simple2 = abs(ba2 - 1.0) < 1e-12

    nc = bass.Bass()
    d_state = nc.declare_dram_parameter("stateT", [SD, BC], bf16, isOutput=False)
    d_w1 = nc.declare_dram_parameter("w1", [H, H], bf16, isOutput=False)
    d_r2 = nc.declare_dram_parameter("r2", [H, H], bf16, isOutput=False)
    d_f1 = nc.declare_dram_parameter("f1s", [T, SD, H], bf16, isOutput=False)
    d_w2 = nc.declare_dram_parameter("w2s", [T, H, H], bf16, isOutput=False)
    d_wo = nc.declare_dram_parameter("wos", [T, H, 2], bf16, isOutput=False)
    d_hc = nc.declare_dram_parameter("hconst", [2, 4], f32, isOutput=False)
    d_om = nc.declare_dram_parameter("out_mean", [1, BC], f32, isOutput=True)
    d_os = nc.declare_dram_parameter("out_std", [1, BC], f32, isOutput=True)

    with ExitStack() as ctx:
        E = ctx.enter_context
        sb_state = E(nc.sbuf_tensor([SD, BC], bf16))
        sb_w1 = E(nc.sbuf_tensor([H, H], bf16))
        sb_r2 = E(nc.sbuf_tensor([H, H], bf16))
        sb_f1 = E(nc.sbuf_tensor([SD, T, H], bf16))
        sb_w2 = E(nc.sbuf_tensor([H, T, H], bf16))
        sb_wo = E(nc.sbuf_tensor([H, T, 2], bf16))
        # head constants, rows (mean,std): cols = sigmoid scale, sigmoid
        # bias, final mul, final add
        sb_hc = E(nc.sbuf_tensor("hc", [2, 4], f32))

        M1 = [[E(nc.sbuf_tensor(f"M1_{i}_{p}", [H, CB], bf16)) for p in range(2)]
              for i in range(G)]
        M2 = [[E(nc.sbuf_tensor(f"M2_{i}_{p}", [H, CB], bf16)) for p in range(2)]
              for i in range(G)]
        S1 = [E(nc.sbuf_tensor(f"S1_{i}", [H, CB], bf16)) for i in range(G)]
        S2 = [E(nc.sbuf_tensor(f"S2_{i}", [H, CB], bf16)) for i in range(G)]
        W1t = [E(nc.sbuf_tensor(f"W1t_{i}", [H, CB], bf16)) for i in range(G)]
        W2t = [E(nc.sbuf_tensor(f"W2t_{i}", [H, CB], bf16)) for i in range(G)]
        t2 = [E(nc.sbuf_tensor(f"t2_{i}", [2, CB], f32)) for i in range(G)]
        sg = [E(nc.sbuf_tensor(f"sg_{i}", [2, CB], f32)) for i in range(G)]
        ou = [E(nc.sbuf_tensor(f"ou_{i}", [2, CB], f32)) for i in range(G)]

        A1p = [E(nc.psum_tensor(f"A1_{i}", [H, CB], f32)) for i in range(G)]
        A2p = [E(nc.psum_tensor(f"A2_{i}", [H, CB], f32)) for i in range(G)]
        AOp = E(nc.psum_tensor("AO", [H, CB], f32))  # chunk slot i: rows 32i..32i+1

        sems = {e: E(nc.semaphore(f"s_{e}")) for e in
                ("pe", "dve", "act", "gps", "sp", "sw", "hw")}

        S = _Sched()

        # ---- plan -----------------------------------------------------
        def dma_in(dst, src, name):
            S.op("sp", lambda nc, eng, d=dst, s=src: eng.dma_start(out=d, in_=s),
                 writes=(name,), dma="hw")

        dma_in(sb_state[:, :], d_state[:, :], "state")
        dma_in(sb_w1[:, :], d_w1[:, :], "w1")
        dma_in(sb_r2[:, :], d_r2[:, :], "r2")
        for t in range(T):
            dma_in(sb_f1[:, t, :], d_f1[t, :, :], f"f1_{t}")
            dma_in(sb_w2[:, t, :], d_w2[t, :, :], f"w2_{t}")
            dma_in(sb_wo[:, t, :], d_wo[t, :, :], f"wo_{t}")

        dma_in(sb_hc[:, :], d_hc[:, :], "hc")

        def emit_layer(C, t, L):
            """Layer L in {1,2} for all chunks of group C at step t,
            sub-stage-major so each engine's stream interleaves chunks."""
            last = t == T - 1
            if L == 1:
                Ap, M, Sp, Wt = A1p, M1, S1, W1t
                an, mn, sn, wn = "A1", "M1", "S1", "W1t"
                simple, ba, a, thr = simple1, ba1, a1, thr1
            else:
                Ap, M, Sp, Wt = A2p, M2, S2, W2t
                an, mn, sn, wn = "A2", "M2", "S2", "W2t"
                simple, ba, a, thr = simple2, ba2, a2, thr2
            c1s = thr * a ** (-t)
            # PE: recurrent + feedforward accumulate
            for c in C:
                i = c % G
                if t > 0:
                    if L == 1:
                        S.op("pe",
                             lambda nc, eng, i=i: nc.tensor.matmul(
                                 A1p[i][:, :], sb_w1[:, :], S1[i][:, :],
                                 start=False, stop=False, skip_group_check=True),
                             reads=("w1", f"S1_{i}"), rmw=(f"A1_{i}",))
                    else:
                        S.op("pe",
                             lambda nc, eng, i=i: nc.tensor.matmul(
                                 A2p[i][:, :], sb_r2[:, :], S2[i][:, :],
                                 start=False, stop=False, skip_group_check=True),
                             reads=("r2", f"S2_{i}"), rmw=(f"A2_{i}",))
                if L == 1:
                    cs = slice(c * CB, (c + 1) * CB)
                    S.op("pe",
                         lambda nc, eng, i=i, t=t, cs=cs, st=(t == 0), la=last:
                             nc.tensor.matmul(A1p[i][:, :], sb_f1[:, t, :],
                                              sb_state[:, cs], start=st, stop=la,
                                              skip_group_check=True),
                         reads=(f"f1_{t}", "state"), rmw=(f"A1_{i}",))
                else:
                    S.op("pe",
                         lambda nc, eng, i=i, t=t, st=(t == 0), la=last:
                             nc.tensor.matmul(A2p[i][:, :], sb_w2[:, t, :],
                                              S1[i][:, :], start=st, stop=la,
                                              skip_group_check=True),
                         reads=(f"w2_{t}", f"S1_{i}"), rmw=(f"A2_{i}",))
            # ACT: drain accumulator into membrane parity buffer
            p = t % 2
            for c in C:
                i = c % G
                S.op("act",
                     lambda nc, eng, i=i, p=p, Ap=Ap, M=M: nc.scalar.activation(
                         out=M[i][p][:, :], in_=Ap[i][:, :], func=Act.Copy),
                     reads=(f"{an}_{i}",), writes=(f"{mn}_{i}_{p}",))
            # DVE: W = (b/a)*M_prev - S_prev ; SWDGE: M += W
            if t > 0:
                for c in C:
                    i = c % G
                    if simple:
                        S.op("dve",
                             lambda nc, eng, i=i, q=1 - p, M=M, Sp=Sp, Wt=Wt:
                                 nc.vector.tensor_tensor(
                                     out=Wt[i][:, :], in0=M[i][q][:, :],
                                     in1=Sp[i][:, :], op=Alu.subtract),
                             reads=(f"{mn}_{i}_{1-p}", f"{sn}_{i}"),
                             writes=(f"{wn}_{i}",))
                    else:
                        S.op("dve",
                             lambda nc, eng, i=i, q=1 - p, M=M, Sp=Sp, Wt=Wt, ba=ba:
                                 nc.vector.scalar_tensor_tensor(
                                     out=Wt[i][:, :], in0=M[i][q][:, :], scalar=ba,
                                     in1=Sp[i][:, :], op0=Alu.mult,
                                     op1=Alu.subtract),
                             reads=(f"{mn}_{i}_{1-p}", f"{sn}_{i}"),
                             writes=(f"{wn}_{i}",))
                for c in C:
                    i = c % G
                    S.op("gps",
                         lambda nc, eng, i=i, p=p, M=M, Wt=Wt: nc.gpsimd.dma_start(
                             out=M[i][p][:, :], in_=Wt[i][:, :], accum_op=Alu.add),
                         reads=(f"{wn}_{i}",), rmw=(f"{mn}_{i}_{p}",), dma="sw")
            # DVE: spike threshold
            for c in C:
                i = c % G
                S.op("dve",
                     lambda nc, eng, i=i, p=p, s1=c1s, s2=c1s / a, M=M, Sp=Sp:
                         nc.vector.tensor_scalar(out=Sp[i][:, :],
                                                 in0=M[i][p][:, :],
                                                 scalar1=s1, scalar2=s2,
                                                 op0=Alu.is_gt, op1=Alu.mult),
                     reads=(f"{mn}_{i}_{p}",), writes=(f"{sn}_{i}",))

        def emit_head_mm(C, t):
            last = t == T - 1
            for c in C:
                i = c % G
                S.op("pe",
                     lambda nc, eng, i=i, t=t, st=(t == 0), la=last:
                         nc.tensor.matmul(AOp[32 * i:32 * i + 2, :],
                                          sb_wo[:, t, :], S2[i][:, :],
                                          start=st, stop=la,
                                          skip_group_check=True),
                     reads=(f"wo_{t}", f"S2_{i}"), rmw=(f"AO_{i}",))

        def emit_tail(c):
            i = c % G
            cs = slice(c * CB, (c + 1) * CB)
            S.op("dve",
                 lambda nc, eng, i=i: nc.vector.tensor_copy(
                     out=t2[i][:, :], in_=AOp[32 * i:32 * i + 2, :]),
                 reads=(f"AO_{i}",), writes=(f"t2_{i}",))
            # rows: (tanh-pre, std-pre).  tanh(x) = 2*sigmoid(2x) - 1
            S.op("act",
                 lambda nc, eng, i=i: nc.scalar.activation(
                     out=sg[i][:, :], in_=t2[i][:, :], func=Act.Sigmoid,
                     scale=sb_hc[:, 0:1], bias=sb_hc[:, 1:2]),
                 reads=(f"t2_{i}", "hc"),
                 writes=(f"sg_{i}",))
            S.op("dve",
                 lambda nc, eng, i=i: nc.vector.tensor_scalar(
                     out=ou[i][:, :], in0=sg[i][:, :],
                     scalar1=sb_hc[:, 2:3], scalar2=sb_hc[:, 3:4],
                     op0=Alu.mult, op1=Alu.add),
                 reads=(f"sg_{i}", "hc"),
                 writes=(f"ou_{i}",))
            S.op("sp", lambda nc, eng, i=i, cs=cs: eng.dma_start(
                     out=d_om[0:1, cs], in_=ou[i][0:1, :]),
                 reads=(f"ou_{i}",), dma="hw")
            S.op("sp", lambda nc, eng, i=i, cs=cs: eng.dma_start(
                     out=d_os[0:1, cs], in_=ou[i][1:2, :]),
                 reads=(f"ou_{i}",), dma="hw")

        for C in GROUPS:
            for t in range(T):
                emit_layer(C, t, 1)
                emit_layer(C, t, 2)
                emit_head_mm(C, t)
            for c in C:
                emit_tail(c)

        # ---- emit -----------------------------------------------------
        block = E(nc.Block())

        def replay(engine_name):
            def body(eng):
                for entry in S.plan[engine_name]:
                    if entry[0] == "wait":
                        _, sem, val = entry
                        eng.wait_ge(sems[sem], val)
                    else:
                        _, fn, sem, val = entry
                        inst = fn(nc, eng)
                        inc = 16 if sem in ("sw", "hw") else 1
                        inst.then_inc(sems[sem], inc)
            return body

        block.tensor(replay("pe"))
        block.vector(replay("dve"))
        block.scalar(replay("act"))
        block.gpsimd(replay("gps"))
        block.sync(replay("sp"))

    return nc


def _prep(scal, w_fc1, w_rec1, w_fc2, w_rec2, w_mean, w_std):
    import ml_dtypes

    a1, thr1 = scal["a1"], scal["thr1"]
    a2, thr2 = scal["a2"], scal["thr2"]
    bf = ml_dtypes.bfloat16
    w1 = (w_rec1.T / thr1).astype(bf)
    r2 = (w_rec2.T / thr2).astype(bf)
    f1s = np.stack([(a1 ** -t) * w_fc1.T for t in range(T)]).astype(bf)
    w2s = np.stack([(a2 ** -t) * (a1 ** (t + 1)) / thr1 * w_fc2.T
                    for t in range(T)]).astype(bf)
    wo = np.concatenate([w_mean, w_std], axis=0).T / (T * thr2)  # [H, 2]
    wos = np.stack([(a2 ** (t + 1)) * wo for t in range(T)]).astype(bf)
    return w1, r2, f1s, w2s, wos


def kernel(state, w_fc1, w_rec1, w_fc2, w_rec2, w_mean, w_std,
           alpha1, beta1, thr1, alpha2, beta2, thr2):
    import ml_dtypes
    from concourse.bass_utils import run_bass_kernel_spmd

    state = np.asarray(state, dtype=np.float32)
    scal = {
        "a1": float(np.clip(np.asarray(alpha1, dtype=np.float64), 1e-6, 1.0)),
        "b1": float(np.clip(np.asarray(beta1, dtype=np.float64), 0.0, 1.0)),
        "thr1": float(np.asarray(thr1, dtype=np.float64)),
        "a2": float(np.clip(np.asarray(alpha2, dtype=np.float64), 1e-6, 1.0)),
        "b2": float(np.clip(np.asarray(beta2, dtype=np.float64), 0.0, 1.0)),
        "thr2": float(np.asarray(thr2, dtype=np.float64)),
    }

    key = tuple(sorted(scal.items()))
    if key not in _CACHE:
        _CACHE[key] = _build(scal)
    nc = _CACHE[key]

    w1, r2, f1s, w2s, wos = _prep(
        scal,
        np.asarray(w_fc1, np.float32), np.asarray(w_rec1, np.float32),
        np.asarray(w_fc2, np.float32), np.asarray(w_rec2, np.float32),
        np.asarray(w_mean, np.float32), np.asarray(w_std, np.float32),
    )
    stateT = state.T.astype(ml_dtypes.bfloat16)  # [6, B_TOTAL]

    in_maps = []
    for c in range(N_CORES):
        in_maps.append({
            "stateT": np.ascontiguousarray(stateT[:, c * BC: (c + 1) * BC]),
            "w1": w1, "r2": r2, "f1s": f1s, "w2s": w2s, "wos": wos,
            "hconst": np.array([[2.0, 0.0, 2.0, -1.0],
                                [1.0, 2.0, 1.9, 0.1]], dtype=np.float32),
        })

    res = run_bass_kernel_spmd(nc, in_maps, core_ids=list(range(N_CORES)),
                               trace=bool(int(os.environ.get("SNN_TRACE", "0"))))
    kernel.last_results = res
    vm = np.concatenate([res.results[c]["out_mean"] for c in range(N_CORES)], axis=1)
    vs = np.concatenate([res.results[c]["out_std"] for c in range(N_CORES)], axis=1)
    return vm.reshape(-1, 1), vs.reshape(-1, 1)


# revision 5
# speedup vs baseline: 1.5302x; 1.5302x over previous
"""Trainium2 Bass kernel for a 2-layer spiking (Synaptic) critic network.

Math (per batch row, T=8 steps, H=128, reset-by-subtract from previous spike):
    cur1 = state @ w_fc1.T
    syn1 = a1*syn1 + cur1 + spk1 @ w_rec1.T ; mem1 = b1*mem1 + syn1 - thr1*spk1_prev
    spk1 = (mem1 > thr1) ; layer 2 analogous with inputs spk1 @ w_fc2.T + spk2 @ w_rec2.T
    out_mean = tanh(mean_t(spk2) @ w_mean.T); out_std = 1.9*sigmoid(.. @ w_std.T + 2) + .1

Formulation (pure data parallel, 8 cores x 8192 rows, hidden on the 128
partitions, batch chunked into CB=512 columns, 16 chunks/core, 3 in flight):

  Work in the a^-t scaled domain so the synaptic accumulator stays resident
  in PSUM for all 8 steps with constant recurrent weights:
    A_t  = sum_{tau<=t} a^-tau (cur_tau + rec-input_tau)   (PSUM, PE-accumulated)
    M_t  = A_t + (b/a)*M_{t-1} - S_{t-1}
    S_t  = (M_t > thr*a^-t) * thr*a^-(t+1)
  The stored spike S carries the a^-(t+1) scale, which makes the recurrent
  matmul weights step-independent; the small feedforward weights get 8
  host-prescaled copies.

Engine assignment (measured per-[128,512]-tile costs in ns):
  PE   : 5 matmuls/chunk-step @216 (rec1, fc1-inject, fc2, rec2, head)
  ACT  : 2 PSUM drains @585 - z1/z2 written DIRECTLY into the membrane tile
  DVE  : W=M-S subtract @337 (STT @604 when b!=a), spike threshold TS @204
  SWDGE: membrane add M += W via gpsimd-issued DMA-accumulate (issue ~630,
         transfer ~390 on the DMA queues, off all compute engines)
  GPSIMD tensor ops are BANNED: the Q7 cores share an SBUF port with the
  vector engine and concurrent gpsimd tensor_tensor degrades every DVE op
  2-3x (measured).  The output head fuses tanh via 2*sigmoid(2x)-1 so one
  [2,512] activation + one per-partition-scalar TS serves both outputs.

Scheduling: plan-then-emit.  Ops are planned chronologically with a small
dependency tracker (per-buffer last-writer/readers -> cross-engine waits as
standalone wait_ge, coalesced via per-engine high-water marks), then each
engine block replays its list.  Raw Bass (no Tile): this walrus build
rejects instructions with >1 attached semaphore wait.
"""

import os
from contextlib import ExitStack

import numpy as np

N_CORES = 8
B_TOTAL = 65536
BC = B_TOTAL // N_CORES  # 8192 rows per core
CB = 512                 # batch-column chunk (one PSUM bank)
NCHUNK = BC // CB        # 16
G = 3                    # chunks interleaved in flight
T = 8
H = 128
SD = 6

GROUPS = [list(range(g, min(g + G, NCHUNK))) for g in range(0, NCHUNK, G)]

_CACHE: dict = {}


class _Sched:
    """Chronological planner: op closures + auto cross-engine waits."""

    ENGINES = ("pe", "dve", "act", "gps", "sp")

    def __init__(self):
        self.plan = {e: [] for e in self.ENGINES}
        self.count = {e: 0 for e in self.ENGINES}   # sem value after op k = k
        self.dma_count = {"sw": 0, "hw": 0}         # completions inc by 16
        self.hwm = {e: {} for e in self.ENGINES}    # per-engine seen sem values
        self.buf = {}                               # name -> {"w": ev, "r": [ev]}

    # events are (sem_name, value_needed)
    def _deps_read(self, name):
        b = self.buf.get(name)
        return [b["w"]] if b and b["w"] else []

    def _deps_write(self, name):
        b = self.buf.get(name)
        if not b:
            return []
        out = [b["w"]] if b["w"] else []
        return out + b["r"]

    def op(self, engine, fn, reads=(), writes=(), rmw=(), dma=None):
        """Plan one op.  engine: issuing engine.  dma: None for compute ops,
        "sw"/"hw" for DMA ops whose completion is a sem inc of 16 on the
        s_sw/s_dma semaphore (issue order still follows `engine`'s stream).
        rmw: buffers both read and written (accumulate)."""
        deps = []
        for n in reads:
            deps += self._deps_read(n)
        for n in writes:
            deps += self._deps_write(n)
        for n in rmw:
            deps += self._deps_write(n)
        # coalesce: keep max value per sem, drop already-satisfied
        need = {}
        for sem, val in deps:
            if sem == engine and dma is None:
                continue  # same-engine program order
            need[sem] = max(need.get(sem, 0), val)
        hw = self.hwm[engine]
        for sem, val in sorted(need.items()):
            if hw.get(sem, 0) < val:
                self.plan[engine].append(("wait", sem, val))
                hw[sem] = val
        if dma is None:
            self.count[engine] += 1
            ev = (engine, self.count[engine])
            self.plan[engine].append(("op", fn, engine, self.count[engine]))
        else:
            self.dma_count[dma] += 16
            ev = (dma, self.dma_count[dma])
            self.plan[engine].append(("op", fn, dma, self.dma_count[dma]))
        for n in writes:
            self.buf[n] = {"w": ev, "r": []}
        for n in rmw:
            b = self.buf.setdefault(n, {"w": None, "r": []})
            b["w"] = ev
            b["r"] = []
        for n in reads:
            b = self.buf.setdefault(n, {"w": None, "r": []})
            b["r"].append(ev)
        return ev


def _build(scal):
    import concourse.bass as bass
    import concourse.mybir as mybir

    a1, b1, thr1 = scal["a1"], scal["b1"], scal["thr1"]
    a2, b2, thr2 = scal["a2"], scal["b2"], scal["thr2"]
    f32 = mybir.dt.float32
    bf16 = mybir.dt.bfloat16
    Alu = mybir.AluOpType
    Act = mybir.ActivationFunctionType

    ba1 = b1 / a1
    ba2 = b2 / a2
    simple1 = abs(ba1 - 1.0) < 1e-12
    simple2 = abs(ba2 - 1.0) < 1e-12

    nc = bass.Bass()
    d_state = nc.declare_dram_parameter("stateT", [SD, BC], bf16, isOutput=False)
    d_w1 = nc.declare_dram_parameter("w1", [H, H], bf16, isOutput=False)
    d_r2 = nc.declare_dram_parameter("r2", [H, H], bf16, isOutput=False)
    d_f1 = nc.declare_dram_parameter("f1s", [T, SD, H], bf16, isOutput=False)
    d_w2 = nc.declare_dram_parameter("w2s", [T, H, H], bf16, isOutput=False)
    d_wo = nc.declare_dram_parameter("wos", [T, H, 2], bf16, isOutput=False)
    d_hc = nc.declare_dram_parameter("hconst", [2, 4], f32, isOutput=False)
    d_om = nc.declare_dram_parameter("out_mean", [1, BC], f32, isOutput=True)
    d_os = nc.declare_dram_parameter("out_std", [1, BC], f32, isOutput=True)

    with ExitStack() as ctx:
        E = ctx.enter_context
        sb_state = E(nc.sbuf_tensor([SD, BC], bf16))
        sb_w1 = E(nc.sbuf_tensor([H, H], bf16))
        sb_r2 = E(nc.sbuf_tensor([H, H], bf16))
        sb_f1 = E(nc.sbuf_tensor([SD, T, H], bf16))
        sb_w2 = E(nc.sbuf_tensor([H, T, H], bf16))
        sb_wo = E(nc.sbuf_tensor([H, T, 2], bf16))
        # head constants, rows (mean,std): cols = sigmoid scale, sigmoid
        # bias, final mul, final add
        sb_hc = E(nc.sbuf_tensor("hc", [2, 4], f32))

        M1 = [[E(nc.sbuf_tensor(f"M1_{i}_{p}", [H, CB], bf16)) for p in range(2)]
              for i in range(G)]
        M2 = [[E(nc.sbuf_tensor(f"M2_{i}_{p}", [H, CB], bf16)) for p in range(2)]
              for i in range(G)]
        S1 = [E(nc.sbuf_tensor(f"S1_{i}", [H, CB], bf16)) for i in range(G)]
        S2 = [E(nc.sbuf_tensor(f"S2_{i}", [H, CB], bf16)) for i in range(G)]
        W1t = [E(nc.sbuf_tensor(f"W1t_{i}", [H, CB], bf16)) for i in range(G)]
        W2t = [E(nc.sbuf_tensor(f"W2t_{i}", [H, CB], bf16)) for i in range(G)]
        t2 = [E(nc.sbuf_tensor(f"t2_{i}", [2, CB], f32)) for i in range(G)]
        sg = [E(nc.sbuf_tensor(f"sg_{i}", [2, CB], f32)) for i in range(G)]
        ou = [E(nc.sbuf_tensor(f"ou_{i}", [2, CB], f32)) for i in range(G)]

        A1p = [E(nc.psum_tensor(f"A1_{i}", [H, CB], f32)) for i in range(G)]
        A2p = [E(nc.psum_tensor(f"A2_{i}", [H, CB], f32)) for i in range(G)]
        AOp = E(nc.psum_tensor("AO", [H, CB], f32))  # chunk slot i: rows 32i..32i+1

        sems = {e: E(nc.semaphore(f"s_{e}")) for e in
                ("pe", "dve", "act", "gps", "sp", "sw", "hw")}

        S = _Sched()

        # ---- plan -----------------------------------------------------
        def dma_in(dst, src, name):
            S.op("sp", lambda nc, eng, d=dst, s=src: eng.dma_start(out=d, in_=s),
                 writes=(name,), dma="hw")

        dma_in(sb_state[:, :], d_state[:, :], "state")
        dma_in(sb_w1[:, :], d_w1[:, :], "w1")
        dma_in(sb_r2[:, :], d_r2[:, :], "r2")
        for t in range(T):
            dma_in(sb_f1[:, t, :], d_f1[t, :, :], f"f1_{t}")
            dma_in(sb_w2[:, t, :], d_w2[t, :, :], f"w2_{t}")
            dma_in(sb_wo[:, t, :], d_wo[t, :, :], f"wo_{t}")

        dma_in(sb_hc[:, :], d_hc[:, :], "hc")

        def emit_layer(C, t, L):
            """Layer L in {1,2} for all chunks of group C at step t,
            sub-stage-major so each engine's stream interleaves chunks."""
            last = t == T - 1
            if L == 1:
                Ap, M, Sp, Wt = A1p, M1, S1, W1t
                an, mn, sn, wn = "A1", "M1", "S1", "W1t"
                simple, ba, a, thr = simple1, ba1, a1, thr1
            else:
                Ap, M, Sp, Wt = A2p, M2, S2, W2t
                an, mn, sn, wn = "A2", "M2", "S2", "W2t"
                simple, ba, a, thr = simple2, ba2, a2, thr2
            c1s = thr * a ** (-t)
            # PE: recurrent + feedforward accumulate
            for c in C:
                i = c % G
                if t > 0:
                    if L == 1:
                        S.op("pe",
                             lambda nc, eng, i=i: nc.tensor.matmul(
                                 A1p[i][:, :], sb_w1[:, :], S1[i][:, :],
                                 start=False, stop=False, skip_group_check=True),
                             reads=("w1", f"S1_{i}"), rmw=(f"A1_{i}",))
                    else:
                        S.op("pe",
                             lambda nc, eng, i=i: nc.tensor.matmul(
                                 A2p[i][:, :], sb_r2[:, :], S2[i][:, :],
                                 start=False, stop=False, skip_group_check=True),
                             reads=("r2", f"S2_{i}"), rmw=(f"A2_{i}",))
                if L == 1:
                    cs = slice(c * CB, (c + 1) * CB)
                    S.op("pe",
                         lambda nc, eng, i=i, t=t, cs=cs, st=(t == 0), la=last:
                             nc.tensor.matmul(A1p[i][:, :], sb_f1[:, t, :],
                                              sb_state[:, cs], start=st, stop=la,
                                              skip_group_check=True),
                         reads=(f"f1_{t}", "state"), rmw=(f"A1_{i}",))
                else:
                    S.op("pe",
                         lambda nc, eng, i=i, t=t, st=(t == 0), la=last:
                             nc.tensor.matmul(A2p[i][:, :], sb_w2[:, t, :],
                                              S1[i][:, :], start=st, stop=la,
                                              skip_group_check=True),
                         reads=(f"w2_{t}", f"S1_{i}"), rmw=(f"A2_{i}",))
            # ACT: drain accumulator into membrane parity buffer
            p = t % 2
            for c in C:
                i = c % G
                S.op("act",
                     lambda nc, eng, i=i, p=p, Ap=Ap, M=M: nc.scalar.activation(
                         out=M[i][p][:, :], in_=Ap[i][:, :], func=Act.Copy),
                     reads=(f"{an}_{i}",), writes=(f"{mn}_{i}_{p}",))
            # DVE: W = (b/a)*M_prev - S_prev ; SWDGE: M += W
            if t > 0:
                for c in C:
                    i = c % G
                    if simple:
                        S.op("dve",
                             lambda nc, eng, i=i, q=1 - p, M=M, Sp=Sp, Wt=Wt:
                                 nc.vector.tensor_tensor(
                                     out=Wt[i][:, :], in0=M[i][q][:, :],
                                     in1=Sp[i][:, :], op=Alu.subtract),
                             reads=(f"{mn}_{i}_{1-p}", f"{sn}_{i}"),
                             writes=(f"{wn}_{i}",))
                    else:
                        S.op("dve",
                             lambda nc, eng, i=i, q=1 - p, M=M, Sp=Sp, Wt=Wt, ba=ba:
                                 nc.vector.scalar_tensor_tensor(
                                     out=Wt[i][:, :], in0=M[i][q][:, :], scalar=ba,
                                     in1=Sp[i][:, :], op0=Alu.mult,
                                     op1=Alu.subtract),
                             reads=(f"{mn}_{i}_{1-p}", f"{sn}_{i}"),
                             writes=(f"{wn}_{i}",))
                for c in C:
                    i = c % G
                    S.op("dve",
                         lambda nc, eng, i=i, p=p, M=M, Wt=Wt:
                             nc.vector.tensor_tensor(
                                 out=M[i][p][:, :], in0=M[i][p][:, :],
                                 in1=Wt[i][:, :], op=Alu.add),
                         reads=(f"{wn}_{i}",), rmw=(f"{mn}_{i}_{p}",))
            # DVE: spike threshold
            for c in C:
                i = c % G
                S.op("dve",
                     lambda nc, eng, i=i, p=p, s1=c1s, s2=c1s / a, M=M, Sp=Sp:
                         nc.vector.tensor_scalar(out=Sp[i][:, :],
                                                 in0=M[i][p][:, :],
                                                 scalar1=s1, scalar2=s2,
                                                 op0=Alu.is_gt, op1=Alu.mult),
                     reads=(f"{mn}_{i}_{p}",), writes=(f"{sn}_{i}",))

        def emit_head_mm(C, t):
            last = t == T - 1
            for c in C:
                i = c % G
                S.op("pe",
                     lambda nc, eng, i=i, t=t, st=(t == 0), la=last:
                         nc.tensor.matmul(AOp[32 * i:32 * i + 2, :],
                                          sb_wo[:, t, :], S2[i][:, :],
                                          start=st, stop=la,
                                          skip_group_check=True),
                     reads=(f"wo_{t}", f"S2_{i}"), rmw=(f"AO_{i}",))

        def emit_tail(c):
            i = c % G
            cs = slice(c * CB, (c + 1) * CB)
            S.op("act",
                 lambda nc, eng, i=i: nc.scalar.activation(
                     out=t2[i][:, :], in_=AOp[32 * i:32 * i + 2, :],
                     func=Act.Copy),
                 reads=(f"AO_{i}",), writes=(f"t2_{i}",))
            # rows: (tanh-pre, std-pre).  tanh(x) = 2*sigmoid(2x) - 1
            S.op("act",
                 lambda nc, eng, i=i: nc.scalar.activation(
                     out=sg[i][:, :], in_=t2[i][:, :], func=Act.Sigmoid,
                     scale=sb_hc[:, 0:1], bias=sb_hc[:, 1:2]),
                 reads=(f"t2_{i}", "hc"),
                 writes=(f"sg_{i}",))
            S.op("dve",
                 lambda nc, eng, i=i: nc.vector.tensor_scalar(
                     out=ou[i][:, :], in0=sg[i][:, :],
                     scalar1=sb_hc[:, 2:3], scalar2=sb_hc[:, 3:4],
                     op0=Alu.mult, op1=Alu.add),
                 reads=(f"sg_{i}", "hc"),
                 writes=(f"ou_{i}",))
            S.op("sp", lambda nc, eng, i=i, cs=cs: eng.dma_start(
                     out=d_om[0:1, cs], in_=ou[i][0:1, :]),
                 reads=(f"ou_{i}",), dma="hw")
            S.op("sp", lambda nc, eng, i=i, cs=cs: eng.dma_start(
                     out=d_os[0:1, cs], in_=ou[i][1:2, :]),
                 reads=(f"ou_{i}",), dma="hw")

        for C in GROUPS:
            for t in range(T):
                emit_layer(C, t, 1)
                emit_layer(C, t, 2)
                emit_head_mm(C, t)
            for c in C:
                emit_tail(c)

        # ---- emit -----------------------------------------------------
        block = E(nc.Block())

        def replay(engine_name):
            def body(eng):
                for entry in S.plan[engine_name]:
                    if entry[0] == "wait":
                        _, sem, val = entry
                        eng.wait_ge(sems[sem], val)
                    else:
                        _, fn, sem, val = entry
                        inst = fn(nc, eng)
                        inc = 16 if sem in ("sw", "hw") else 1
                        inst.then_inc(sems[sem], inc)
            return body

        block.tensor(replay("pe"))
        block.vector(replay("dve"))
        block.scalar(replay("act"))
        block.gpsimd(replay("gps"))
        block.sync(replay("sp"))

    return nc


def _prep(scal, w_fc1, w_rec1, w_fc2, w_rec2, w_mean, w_std):
    import ml_dtypes

    a1, thr1 = scal["a1"], scal["thr1"]
    a2, thr2 = scal["a2"], scal["thr2"]
    bf = ml_dtypes.bfloat16
    w1 = (w_rec1.T / thr1).astype(bf)
    r2 = (w_rec2.T / thr2).astype(bf)
    f1s = np.stack([(a1 ** -t) * w_fc1.T for t in range(T)]).astype(bf)
    w2s = np.stack([(a2 ** -t) * (a1 ** (t + 1)) / thr1 * w_fc2.T
                    for t in range(T)]).astype(bf)
    wo = np.concatenate([w_mean, w_std], axis=0).T / (T * thr2)  # [H, 2]
    wos = np.stack([(a2 ** (t + 1)) * wo for t in range(T)]).astype(bf)
    return w1, r2, f1s, w2s, wos


def kernel(state, w_fc1, w_rec1, w_fc2, w_rec2, w_mean, w_std,
           alpha1, beta1, thr1, alpha2, beta2, thr2):
    import ml_dtypes
    from concourse.bass_utils import run_bass_kernel_spmd

    state = np.asarray(state, dtype=np.float32)
    scal = {
        "a1": float(np.clip(np.asarray(alpha1, dtype=np.float64), 1e-6, 1.0)),
        "b1": float(np.clip(np.asarray(beta1, dtype=np.float64), 0.0, 1.0)),
        "thr1": float(np.asarray(thr1, dtype=np.float64)),
        "a2": float(np.clip(np.asarray(alpha2, dtype=np.float64), 1e-6, 1.0)),
        "b2": float(np.clip(np.asarray(beta2, dtype=np.float64), 0.0, 1.0)),
        "thr2": float(np.asarray(thr2, dtype=np.float64)),
    }

    key = tuple(sorted(scal.items()))
    if key not in _CACHE:
        _CACHE[key] = _build(scal)
    nc = _CACHE[key]

    w1, r2, f1s, w2s, wos = _prep(
        scal,
        np.asarray(w_fc1, np.float32), np.asarray(w_rec1, np.float32),
        np.asarray(w_fc2, np.float32), np.asarray(w_rec2, np.float32),
        np.asarray(w_mean, np.float32), np.asarray(w_std, np.float32),
    )
    stateT = state.T.astype(ml_dtypes.bfloat16)  # [6, B_TOTAL]

    in_maps = []
    for c in range(N_CORES):
        in_maps.append({
            "stateT": np.ascontiguousarray(stateT[:, c * BC: (c + 1) * BC]),
            "w1": w1, "r2": r2, "f1s": f1s, "w2s": w2s, "wos": wos,
            "hconst": np.array([[2.0, 0.0, 2.0, -1.0],
                                [1.0, 2.0, 1.9, 0.1]], dtype=np.float32),
        })

    res = run_bass_kernel_spmd(nc, in_maps, core_ids=list(range(N_CORES)),
                               trace=bool(int(os.environ.get("SNN_TRACE", "0"))))
    kernel.last_results = res
    vm = np.concatenate([res.results[c]["out_mean"] for c in range(N_CORES)], axis=1)
    vs = np.concatenate([res.results[c]["out_std"] for c in range(N_CORES)], axis=1)
    return vm.reshape(-1, 1), vs.reshape(-1, 1)


# revision 7
# speedup vs baseline: 2.0239x; 1.3227x over previous
"""Trainium2 Bass kernel for a 2-layer spiking (Synaptic) critic network.

Math (per batch row, T=8 steps, H=128, reset-by-subtract from previous spike):
    cur1 = state @ w_fc1.T
    syn1 = a1*syn1 + cur1 + spk1 @ w_rec1.T ; mem1 = b1*mem1 + syn1 - thr1*spk1_prev
    spk1 = (mem1 > thr1) ; layer 2 analogous with inputs spk1 @ w_fc2.T + spk2 @ w_rec2.T
    out_mean = tanh(mean_t(spk2) @ w_mean.T); out_std = 1.9*sigmoid(.. @ w_std.T + 2) + .1

Formulation (pure data parallel, 8 cores x 8192 rows, hidden on the 128
partitions, batch chunked into CB=512 columns, 16 chunks/core, 3 in flight):

  Work in the a^-t scaled domain so the synaptic accumulator stays resident
  in PSUM for all 8 steps with constant recurrent weights:
    A_t  = sum_{tau<=t} a^-tau (cur_tau + rec-input_tau)   (PSUM, PE-accumulated)
    M_t  = A_t + (b/a)*M_{t-1} - S_{t-1}
    S_t  = (M_t > thr*a^-t) * thr*a^-(t+1)
  The stored spike S carries the a^-(t+1) scale, which makes the recurrent
  matmul weights step-independent; the small feedforward weights get 8
  host-prescaled copies.

Engine assignment (measured per-[128,512]-tile costs in ns):
  PE   : 5 matmuls/chunk-step @216 (rec1, fc1-inject, fc2, rec2, head)
  ACT  : 2 PSUM drains @585 - z1/z2 written DIRECTLY into the membrane tile
  DVE  : W=M-S subtract @337 (STT @604 when b!=a), spike threshold TS @204
  SWDGE: membrane add M += W via gpsimd-issued DMA-accumulate (issue ~630,
         transfer ~390 on the DMA queues, off all compute engines)
  GPSIMD tensor ops are BANNED: the Q7 cores share an SBUF port with the
  vector engine and concurrent gpsimd tensor_tensor degrades every DVE op
  2-3x (measured).  The output head fuses tanh via 2*sigmoid(2x)-1 so one
  [2,512] activation + one per-partition-scalar TS serves both outputs.

Scheduling: plan-then-emit.  Ops are planned chronologically with a small
dependency tracker (per-buffer last-writer/readers -> cross-engine waits as
standalone wait_ge, coalesced via per-engine high-water marks), then each
engine block replays its list.  Raw Bass (no Tile): this walrus build
rejects instructions with >1 attached semaphore wait.
"""

import os
from contextlib import ExitStack

import numpy as np

N_CORES = 8
B_TOTAL = 65536
BC = B_TOTAL // N_CORES  # 8192 rows per core
CB = 512                 # batch-column chunk (one PSUM bank)
NCHUNK = BC // CB        # 16
G = 3                    # chunks interleaved in flight
T = 8
H = 128
SD = 6

GROUPS = [list(range(g, min(g + G, NCHUNK))) for g in range(0, NCHUNK, G)]

_CACHE: dict = {}


class _Sched:
    """Chronological planner: op closures + auto cross-engine waits."""

    ENGINES = ("pe", "dve", "act", "gps", "sp")

    def __init__(self):
        self.plan = {e: [] for e in self.ENGINES}
        self.count = {e: 0 for e in self.ENGINES}   # sem value after op k = k
        self.dma_count = {"sw": 0, "hw": 0}         # completions inc by 16
        self.hwm = {e: {} for e in self.ENGINES}    # per-engine seen sem values
        self.buf = {}                               # name -> {"w": ev, "r": [ev]}

    # events are (sem_name, value_needed)
    def _deps_read(self, name):
        b = self.buf.get(name)
        return [b["w"]] if b and b["w"] else []

    def _deps_write(self, name):
        b = self.buf.get(name)
        if not b:
            return []
        out = [b["w"]] if b["w"] else []
        return out + b["r"]

    def op(self, engine, fn, reads=(), writes=(), rmw=(), dma=None):
        """Plan one op.  engine: issuing engine.  dma: None for compute ops,
        "sw"/"hw" for DMA ops whose completion is a sem inc of 16 on the
        s_sw/s_dma semaphore (issue order still follows `engine`'s stream).
        rmw: buffers both read and written (accumulate)."""
        deps = []
        for n in reads:
            deps += self._deps_read(n)
        for n in writes:
            deps += self._deps_write(n)
        for n in rmw:
            deps += self._deps_write(n)
        # coalesce: keep max value per sem, drop already-satisfied
        need = {}
        for sem, val in deps:
            if sem == engine and dma is None:
                continue  # same-engine program order
            need[sem] = max(need.get(sem, 0), val)
        hw = self.hwm[engine]
        for sem, val in sorted(need.items()):
            if hw.get(sem, 0) < val:
                self.plan[engine].append(("wait", sem, val))
                hw[sem] = val
        if dma is None:
            self.count[engine] += 1
            ev = (engine, self.count[engine])
            self.plan[engine].append(("op", fn, engine, self.count[engine]))
        else:
            self.dma_count[dma] += 16
            ev = (dma, self.dma_count[dma])
            self.plan[engine].append(("op", fn, dma, self.dma_count[dma]))
        for n in writes:
            self.buf[n] = {"w": ev, "r": []}
        for n in rmw:
            b = self.buf.setdefault(n, {"w": None, "r": []})
            b["w"] = ev
            b["r"] = []
        for n in reads:
            b = self.buf.setdefault(n, {"w": None, "r": []})
            b["r"].append(ev)
        return ev


def _build(scal):
    import concourse.bass as bass
    import concourse.mybir as mybir

    a1, b1, thr1 = scal["a1"], scal["b1"], scal["thr1"]
    a2, b2, thr2 = scal["a2"], scal["b2"], scal["thr2"]
    f32 = mybir.dt.float32
    bf16 = mybir.dt.bfloat16
    Alu = mybir.AluOpType
    Act = mybir.ActivationFunctionType

    ba1 = b1 / a1
    ba2 = b2 / a2
    simple1 = abs(ba1 - 1.0) < 1e-12
    simple2 = abs(ba2 - 1.0) < 1e-12

    nc = bass.Bass()
    d_state = nc.declare_dram_parameter("stateT", [SD, BC], bf16, isOutput=False)
    d_w1 = nc.declare_dram_parameter("w1", [H, H], bf16, isOutput=False)
    d_r2 = nc.declare_dram_parameter("r2", [H, H], bf16, isOutput=False)
    d_f1 = nc.declare_dram_parameter("f1s", [T, SD, H], bf16, isOutput=False)
    d_w2 = nc.declare_dram_parameter("w2s", [T, H, H], bf16, isOutput=False)
    d_wo = nc.declare_dram_parameter("wos", [T, H, 2], bf16, isOutput=False)
    d_hc = nc.declare_dram_parameter("hconst", [2, 4], f32, isOutput=False)
    d_om = nc.declare_dram_parameter("out_mean", [1, BC], f32, isOutput=True)
    d_os = nc.declare_dram_parameter("out_std", [1, BC], f32, isOutput=True)

    with ExitStack() as ctx:
        E = ctx.enter_context
        sb_state = E(nc.sbuf_tensor([SD, BC], bf16))
        sb_w1 = E(nc.sbuf_tensor([H, H], bf16))
        sb_r2 = E(nc.sbuf_tensor([H, H], bf16))
        sb_f1 = E(nc.sbuf_tensor([SD, T, H], bf16))
        sb_w2 = E(nc.sbuf_tensor([H, T, H], bf16))
        sb_wo = E(nc.sbuf_tensor([H, T, 2], bf16))
        # head constants, rows (mean,std): cols = sigmoid scale, sigmoid
        # bias, final mul, final add
        sb_hc = E(nc.sbuf_tensor("hc", [2, 4], f32))

        M1 = [E(nc.sbuf_tensor(f"M1_{i}", [H, CB], bf16)) for i in range(G)]
        M2 = [E(nc.sbuf_tensor(f"M2_{i}", [H, CB], bf16)) for i in range(G)]
        Z1 = [E(nc.sbuf_tensor(f"Z1_{i}", [H, CB], bf16)) for i in range(G)]
        Z2 = [E(nc.sbuf_tensor(f"Z2_{i}", [H, CB], bf16)) for i in range(G)]
        S1 = [E(nc.sbuf_tensor(f"S1_{i}", [H, CB], bf16)) for i in range(G)]
        S2 = [E(nc.sbuf_tensor(f"S2_{i}", [H, CB], bf16)) for i in range(G)]
        W1t = [E(nc.sbuf_tensor(f"W1t_{i}", [H, CB], bf16)) for i in range(G)]
        W2t = [E(nc.sbuf_tensor(f"W2t_{i}", [H, CB], bf16)) for i in range(G)]
        t2 = [E(nc.sbuf_tensor(f"t2_{i}", [2, CB], f32)) for i in range(G)]
        sg = [E(nc.sbuf_tensor(f"sg_{i}", [2, CB], f32)) for i in range(G)]
        ou = [E(nc.sbuf_tensor(f"ou_{i}", [2, CB], f32)) for i in range(G)]

        A1p = [E(nc.psum_tensor(f"A1_{i}", [H, CB], f32)) for i in range(G)]
        A2p = [E(nc.psum_tensor(f"A2_{i}", [H, CB], f32)) for i in range(G)]
        AOp = E(nc.psum_tensor("AO", [H, CB], f32))  # chunk slot i: rows 32i..32i+1

        sems = {e: E(nc.semaphore(f"s_{e}")) for e in
                ("pe", "dve", "act", "gps", "sp", "sw", "hw")}

        S = _Sched()

        # ---- plan -----------------------------------------------------
        def dma_in(dst, src, name):
            S.op("sp", lambda nc, eng, d=dst, s=src: eng.dma_start(out=d, in_=s),
                 writes=(name,), dma="hw")

        dma_in(sb_state[:, :], d_state[:, :], "state")
        dma_in(sb_w1[:, :], d_w1[:, :], "w1")
        dma_in(sb_r2[:, :], d_r2[:, :], "r2")
        for t in range(T):
            dma_in(sb_f1[:, t, :], d_f1[t, :, :], f"f1_{t}")
            dma_in(sb_w2[:, t, :], d_w2[t, :, :], f"w2_{t}")
            dma_in(sb_wo[:, t, :], d_wo[t, :, :], f"wo_{t}")

        dma_in(sb_hc[:, :], d_hc[:, :], "hc")

        for k in range(12):
            S.op("pe",
                 lambda nc, eng: nc.tensor.matmul(
                     A1p[0][:, :], sb_f1[:, 0, :], sb_state[:, 0:CB],
                     start=True, stop=True, skip_group_check=True),
                 reads=("f1_0", "state"), rmw=("A1_0",))

        def emit_layer(C, t, L):
            """Layer L in {1,2} for all chunks of group C at step t,
            sub-stage-major so each engine's stream interleaves chunks."""
            last = t == T - 1
            if L == 1:
                Ap, M, Z, Sp, Wt = A1p, M1, Z1, S1, W1t
                an, mn, zn, sn, wn = "A1", "M1", "Z1", "S1", "W1t"
                simple, ba, a, thr = simple1, ba1, a1, thr1
            else:
                Ap, M, Z, Sp, Wt = A2p, M2, Z2, S2, W2t
                an, mn, zn, sn, wn = "A2", "M2", "Z2", "S2", "W2t"
                simple, ba, a, thr = simple2, ba2, a2, thr2
            c1s = thr * a ** (-t)
            # PE: recurrent + feedforward accumulate
            for c in C:
                i = c % G
                if t > 0:
                    if L == 1:
                        S.op("pe",
                             lambda nc, eng, i=i: nc.tensor.matmul(
                                 A1p[i][:, :], sb_w1[:, :], S1[i][:, :],
                                 start=False, stop=False, skip_group_check=True),
                             reads=("w1", f"S1_{i}"), rmw=(f"A1_{i}",))
                    else:
                        S.op("pe",
                             lambda nc, eng, i=i: nc.tensor.matmul(
                                 A2p[i][:, :], sb_r2[:, :], S2[i][:, :],
                                 start=False, stop=False, skip_group_check=True),
                             reads=("r2", f"S2_{i}"), rmw=(f"A2_{i}",))
                if L == 1:
                    cs = slice(c * CB, (c + 1) * CB)
                    S.op("pe",
                         lambda nc, eng, i=i, t=t, cs=cs, st=(t == 0), la=last:
                             nc.tensor.matmul(A1p[i][:, :], sb_f1[:, t, :],
                                              sb_state[:, cs], start=st, stop=la,
                                              skip_group_check=True),
                         reads=(f"f1_{t}", "state"), rmw=(f"A1_{i}",))
                else:
                    S.op("pe",
                         lambda nc, eng, i=i, t=t, st=(t == 0), la=last:
                             nc.tensor.matmul(A2p[i][:, :], sb_w2[:, t, :],
                                              S1[i][:, :], start=st, stop=la,
                                              skip_group_check=True),
                         reads=(f"w2_{t}", f"S1_{i}"), rmw=(f"A2_{i}",))
            # ACT: drain accumulator (t=0: straight into M, else into Z)
            for c in C:
                i = c % G
                if t == 0:
                    S.op("act",
                         lambda nc, eng, i=i, Ap=Ap, M=M: nc.scalar.activation(
                             out=M[i][:, :], in_=Ap[i][:, :], func=Act.Copy),
                         reads=(f"{an}_{i}",), writes=(f"{mn}_{i}",))
                else:
                    S.op("act",
                         lambda nc, eng, i=i, Ap=Ap, Z=Z: nc.scalar.activation(
                             out=Z[i][:, :], in_=Ap[i][:, :], func=Act.Copy),
                         reads=(f"{an}_{i}",), writes=(f"{zn}_{i}",))
            # DVE: W = (b/a)*M_prev - S_prev ; M = Z + W
            if t > 0:
                for c in C:
                    i = c % G
                    if simple:
                        S.op("dve",
                             lambda nc, eng, i=i, M=M, Sp=Sp, Wt=Wt:
                                 nc.vector.tensor_tensor(
                                     out=Wt[i][:, :], in0=M[i][:, :],
                                     in1=Sp[i][:, :], op=Alu.subtract),
                             reads=(f"{mn}_{i}", f"{sn}_{i}"),
                             writes=(f"{wn}_{i}",))
                    else:
                        S.op("dve",
                             lambda nc, eng, i=i, M=M, Sp=Sp, Wt=Wt, ba=ba:
                                 nc.vector.scalar_tensor_tensor(
                                     out=Wt[i][:, :], in0=M[i][:, :], scalar=ba,
                                     in1=Sp[i][:, :], op0=Alu.mult,
                                     op1=Alu.subtract),
                             reads=(f"{mn}_{i}", f"{sn}_{i}"),
                             writes=(f"{wn}_{i}",))
                for c in C:
                    i = c % G
                    S.op("dve",
                         lambda nc, eng, i=i, M=M, Z=Z, Wt=Wt:
                             nc.vector.tensor_tensor(
                                 out=M[i][:, :], in0=Z[i][:, :],
                                 in1=Wt[i][:, :], op=Alu.add),
                         reads=(f"{zn}_{i}", f"{wn}_{i}"), writes=(f"{mn}_{i}",))
            # DVE: spike threshold
            for c in C:
                i = c % G
                S.op("dve",
                     lambda nc, eng, i=i, s1=c1s, s2=c1s / a, M=M, Sp=Sp:
                         nc.vector.tensor_scalar(out=Sp[i][:, :],
                                                 in0=M[i][:, :],
                                                 scalar1=s1, scalar2=s2,
                                                 op0=Alu.is_gt, op1=Alu.mult),
                     reads=(f"{mn}_{i}",), writes=(f"{sn}_{i}",))

        def emit_head_mm(C, t):
            last = t == T - 1
            for c in C:
                i = c % G
                S.op("pe",
                     lambda nc, eng, i=i, t=t, st=(t == 0), la=last:
                         nc.tensor.matmul(AOp[32 * i:32 * i + 2, :],
                                          sb_wo[:, t, :], S2[i][:, :],
                                          start=st, stop=la,
                                          skip_group_check=True),
                     reads=(f"wo_{t}", f"S2_{i}"), rmw=(f"AO_{i}",))

        def emit_tail(c):
            i = c % G
            cs = slice(c * CB, (c + 1) * CB)
            S.op("act",
                 lambda nc, eng, i=i: nc.scalar.activation(
                     out=t2[i][:, :], in_=AOp[32 * i:32 * i + 2, :],
                     func=Act.Copy),
                 reads=(f"AO_{i}",), writes=(f"t2_{i}",))
            # rows: (tanh-pre, std-pre).  tanh(x) = 2*sigmoid(2x) - 1
            S.op("act",
                 lambda nc, eng, i=i: nc.scalar.activation(
                     out=sg[i][:, :], in_=t2[i][:, :], func=Act.Sigmoid,
                     scale=sb_hc[:, 0:1], bias=sb_hc[:, 1:2]),
                 reads=(f"t2_{i}", "hc"),
                 writes=(f"sg_{i}",))
            S.op("dve",
                 lambda nc, eng, i=i: nc.vector.tensor_scalar(
                     out=ou[i][:, :], in0=sg[i][:, :],
                     scalar1=sb_hc[:, 2:3], scalar2=sb_hc[:, 3:4],
                     op0=Alu.mult, op1=Alu.add),
                 reads=(f"sg_{i}", "hc"),
                 writes=(f"ou_{i}",))
            S.op("sp", lambda nc, eng, i=i, cs=cs: eng.dma_start(
                     out=d_om[0:1, cs], in_=ou[i][0:1, :]),
                 reads=(f"ou_{i}",), dma="hw")
            S.op("sp", lambda nc, eng, i=i, cs=cs: eng.dma_start(
                     out=d_os[0:1, cs], in_=ou[i][1:2, :]),
                 reads=(f"ou_{i}",), dma="hw")

        for C in GROUPS:
            for t in range(T):
                emit_layer(C, t, 1)
                if t >= 1:
                    emit_head_mm(C, t - 1)
                emit_layer(C, t, 2)
            emit_head_mm(C, T - 1)
            for c in C:
                emit_tail(c)

        # ---- emit -----------------------------------------------------
        block = E(nc.Block())

        def replay(engine_name):
            def body(eng):
                for entry in S.plan[engine_name]:
                    if entry[0] == "wait":
                        _, sem, val = entry
                        eng.wait_ge(sems[sem], val)
                    else:
                        _, fn, sem, val = entry
                        inst = fn(nc, eng)
                        inc = 16 if sem in ("sw", "hw") else 1
                        inst.then_inc(sems[sem], inc)
            return body

        block.tensor(replay("pe"))
        block.vector(replay("dve"))
        block.scalar(replay("act"))
        block.gpsimd(replay("gps"))
        block.sync(replay("sp"))

    return nc


def _prep(scal, w_fc1, w_rec1, w_fc2, w_rec2, w_mean, w_std):
    import ml_dtypes

    a1, thr1 = scal["a1"], scal["thr1"]
    a2, thr2 = scal["a2"], scal["thr2"]
    bf = ml_dtypes.bfloat16
    w1 = (w_rec1.T / thr1).astype(bf)
    r2 = (w_rec2.T / thr2).astype(bf)
    f1s = np.stack([(a1 ** -t) * w_fc1.T for t in range(T)]).astype(bf)
    w2s = np.stack([(a2 ** -t) * (a1 ** (t + 1)) / thr1 * w_fc2.T
                    for t in range(T)]).astype(bf)
    wo = np.concatenate([w_mean, w_std], axis=0).T / (T * thr2)  # [H, 2]
    wos = np.stack([(a2 ** (t + 1)) * wo for t in range(T)]).astype(bf)
    return w1, r2, f1s, w2s, wos


def kernel(state, w_fc1, w_rec1, w_fc2, w_rec2, w_mean, w_std,
           alpha1, beta1, thr1, alpha2, beta2, thr2):
    import ml_dtypes
    from concourse.bass_utils import run_bass_kernel_spmd

    state = np.asarray(state, dtype=np.float32)
    scal = {
        "a1": float(np.clip(np.asarray(alpha1, dtype=np.float64), 1e-6, 1.0)),
        "b1": float(np.clip(np.asarray(beta1, dtype=np.float64), 0.0, 1.0)),
        "thr1": float(np.asarray(thr1, dtype=np.float64)),
        "a2": float(np.clip(np.asarray(alpha2, dtype=np.float64), 1e-6, 1.0)),
        "b2": float(np.clip(np.asarray(beta2, dtype=np.float64), 0.0, 1.0)),
        "thr2": float(np.asarray(thr2, dtype=np.float64)),
    }

    key = tuple(sorted(scal.items()))
    if key not in _CACHE:
        _CACHE[key] = _build(scal)
    nc = _CACHE[key]

    w1, r2, f1s, w2s, wos = _prep(
        scal,
        np.asarray(w_fc1, np.float32), np.asarray(w_rec1, np.float32),
        np.asarray(w_fc2, np.float32), np.asarray(w_rec2, np.float32),
        np.asarray(w_mean, np.float32), np.asarray(w_std, np.float32),
    )
    stateT = state.T.astype(ml_dtypes.bfloat16)  # [6, B_TOTAL]

    in_maps = []
    for c in range(N_CORES):
        in_maps.append({
            "stateT": np.ascontiguousarray(stateT[:, c * BC: (c + 1) * BC]),
            "w1": w1, "r2": r2, "f1s": f1s, "w2s": w2s, "wos": wos,
            "hconst": np.array([[2.0, 0.0, 2.0, -1.0],
                                [1.0, 2.0, 1.9, 0.1]], dtype=np.float32),
        })

    res = run_bass_kernel_spmd(nc, in_maps, core_ids=list(range(N_CORES)),
                               trace=bool(int(os.environ.get("SNN_TRACE", "0"))))
    kernel.last_results = res
    vm = np.concatenate([res.results[c]["out_mean"] for c in range(N_CORES)], axis=1)
    vs = np.concatenate([res.results[c]["out_std"] for c in range(N_CORES)], axis=1)
    return vm.reshape(-1, 1), vs.reshape(-1, 1)
